# revision 1
# baseline (speedup 1.0000x reference)
"""Trainium2 Bass kernel for nn_ALNet (adaptive linear network forward).

Math: vals = x @ W + b  ([65536,256] @ [256,128] + [128]), then a 7-level
alternating min/max pairwise tree over the 128 leaf columns -> [B, 1].

Strategy (8 NeuronCores, data-parallel over the batch; per-core shard 8192):
  W-stationary matmul orientation. Per core the PE computes
  psum[leaf, batch] = Wh^T @ xh (two K-halves accumulating), so the small
  W[128,128] half is the stationary operand and the 8192 batch columns
  stream through -- the per-matmul LDWEIGHTS cost drops from one 128-row
  load per 128 batch rows (the old x-stationary layout, ~50% PE overhead)
  to one per 512-col PSUM bank (~20%).  Bias becomes a per-PARTITION
  constant in this orientation, so it rides the ACT eviction for free
  (activation Identity with a [128,1] bias AP) and the old bias-seed
  matmuls (~8k PE cycles) disappear.

  Tree: leaves are host-permuted into bit-reversed order, so the deepest
  level pairs leaves (p, p+64) across partitions: DVE computes
  L1 = min(vb[0:64], vb[64:128]) straight on the evicted fp16 data.  The
  halved [64, batch] result is transposed back to [batch, leaf] by PE
  transpose matmuls (64 blocks of [64,128] -> [128,64], identity rhs), and
  the remaining 6 levels run on the free dim at full 128-lane DVE
  utilization, exactly like the old kernel's tree.

  DMA: x is host-prepped fp16 in a chunk-major layout [4 super-chunks x
  2 K-halves x 128 x 2048] so every x load is one fully contiguous 512KB
  read; K-half-0 loads issue on the sync ring and K-half-1 on the gpsimd
  ring so both halves of a chunk land in parallel.

  Output staged as [128, 64] f32 (out[p, c] = batch row 128*c+p),
  de-interleaved on the host.
"""

import numpy as np

try:
    import concourse.bass as bass
except ImportError:  # pragma: no cover
    import sys

    sys.path.insert(0, "/opt/trn_rl_repo")
    import concourse.bass as bass

import concourse.mybir as mybir
import concourse.tile as tile
from concourse import bacc
from concourse.bass_utils import run_bass_kernel_spmd

F32 = mybir.dt.float32
F16 = mybir.dt.float16

B, F, NL = 65536, 256, 128
NCORES = 8
BS = B // NCORES  # 8192 batch rows per core

SUP = 2048  # batch cols per super-chunk (one contiguous 512KB DMA per K-half)
NSUP = BS // SUP  # 4
SUB = 1024  # batch cols per PSUM group (2 banks)
NSUB = SUP // SUB  # 2

# Tree ops, deepest level first (palindrome list: min,max,min,max,min,max,min)
_TREE_OPS = [
    mybir.AluOpType.min if i % 2 == 0 else mybir.AluOpType.max for i in range(7)
]


def _bitrev7_perm() -> np.ndarray:
    perm = np.zeros(NL, dtype=np.int64)
    for p in range(NL):
        r = 0
        for k in range(7):
            r |= ((p >> k) & 1) << (6 - k)
        perm[p] = r
    return perm


def build_nc(bs: int = BS):
    nc = bacc.Bacc(None)
    # xw0: [x K-half-0 cols 0:4096 | W K-half-0 (128) | bias (1) | pad]
    # xw1: [x K-half-1 cols 0:4096 | W K-half-1 (128) | pad]
    # xr:  rows 0:128 = x K-half-0 cols 4096:8192, rows 128:256 = K-half-1
    xw0 = nc.declare_dram_parameter("xw0", [128, 4352], F16, isOutput=False)
    xw1 = nc.declare_dram_parameter("xw1", [128, 4352], F16, isOutput=False)
    xr = nc.declare_dram_parameter("xr", [256, 4096], F16, isOutput=False)
    ncols = bs // 128  # 64
    out = nc.declare_dram_parameter("out", [128, ncols], F32, isOutput=True)

    # psum chunks: 6 x 1024 then 4 x 512 (short tail chain)
    chunks = []
    c0 = 0
    for ln in [1024] * 6 + [512] * 4:
        chunks.append((c0, ln))
        c0 += ln
    assert c0 == bs
    DVE_EVICT = {0, 1, 8}  # eviction on DVE for these; ACT for the rest

    with tile.TileContext(nc, pool_alloc_mode="queue") as tc:
        with (
            tc.tile_pool(name="xin", bufs=1) as xpool,
            tc.tile_pool(name="psum", bufs=3, space=bass.MemorySpace.PSUM) as ppool,
            tc.tile_pool(name="psums", bufs=2, space=bass.MemorySpace.PSUM) as ppool_s,
            tc.tile_pool(name="sb", bufs=1) as spool,
        ):
            # one DMA per K-half pair: W and bias ride the first x loads, so
            # only 4 large aligned DMAs feed the whole kernel
            x00 = xpool.tile([128, 4352], F16, tag="x00")
            x01 = xpool.tile([128, 4096], F16, tag="x01")
            x10 = xpool.tile([128, 4352], F16, tag="x10")
            x11 = xpool.tile([128, 4096], F16, tag="x11")
            nc.sync.dma_start(out=x00[:], in_=xw0[:])
            nc.scalar.dma_start(out=x10[:], in_=xw1[:])
            nc.sync.dma_start(out=x01[:], in_=xr[0:128, :])
            nc.scalar.dma_start(out=x11[:], in_=xr[128:256, :])
            w0t = x00[:, 4096:4224]
            w1t = x10[:, 4096:4224]
            bch = x00[:, 4224:4225]
            xh = {0: (x00, x01), 1: (x10, x11)}

            def xslice(h, col, ln):
                t = xh[h][col // 4096]
                o = col % 4096
                return t[:, o : o + ln]

            bcf = spool.tile([128, 1], F32, tag="bcf")
            nc.vector.tensor_copy(bcf[:], bch)
            bct = bcf[:]

            # flat SBUF intermediates
            vb = spool.tile([128, bs], F16, tag="vb")  # [leaf, batch]
            vt = spool.tile([128, bs], F16, tag="vt")  # [batch_p, blk*128]
            l1 = spool.tile([128, bs // 2], F16, tag="l1")
            lvl_tiles = []
            w = 32
            n = bs // 4
            while w >= 2:
                lvl_tiles.append(
                    spool.tile([128, n], F16, tag=f"lv{w}", name=f"lv{w}")
                )
                w //= 2
                n //= 2
            ost = spool.tile([128, ncols], F32, tag="ost")

            # PE p-state warmup: garbage matmuls with no input deps keep the
            # PE streaming from the preamble until real x data lands
            garb = spool.tile([128, 512], F16, tag="garb")
            nc.gpsimd.memset(garb[:], 0.0)

            pss = {}
            for c, (col0, ln) in enumerate(chunks):
                pool = ppool if ln == 1024 else ppool_s
                pss[c] = pool.tile([128, ln], F32, tag=f"ps{ln}", name=f"ps_{c}")
            for i in range(14):
                nc.tensor.matmul(
                    pss[0][:, 0:512], garb[:, 0:128], garb[:],
                    start=True, stop=True,
                )

            def evict(c):
                col0, ln = chunks[c]
                slc = slice(col0, col0 + ln)
                if c in DVE_EVICT:
                    nc.vector.tensor_scalar(
                        out=vb[:, slc], in0=pss[c][:], scalar1=bct,
                        scalar2=None, op0=mybir.AluOpType.add,
                    )
                else:
                    nc.scalar.activation(
                        vb[:, slc], pss[c][:],
                        mybir.ActivationFunctionType.Identity,
                        bias=bct, scale=1.0,
                    )

            def tr_l1(eng, col0, ln):
                # NOTE: XBAR transposes must never overlap in time (shared
                # hw resource) -- keep them all on one queue
                eng.dma_start(
                    out=vt[:, col0 : col0 + ln].rearrange(
                        "p (blk l) -> p blk l", l=128
                    ),
                    in_=vb[:, col0 : col0 + ln],
                    transpose=True,
                )
                rr = vt[:, col0 : col0 + ln].rearrange(
                    "p (blk two h) -> p blk two h", two=2, h=64
                )
                nc.vector.tensor_tensor(
                    out=l1[:, col0 // 2 : (col0 + ln) // 2].rearrange(
                        "p (blk h) -> p blk h", h=64
                    ),
                    in0=rr[:, :, 0, :], in1=rr[:, :, 1, :], op=_TREE_OPS[0],
                )

            def tree(col0, ncols_span):
                # levels 2..7 for batch cols [col0, col0+ncols_span)
                cur = l1[:, col0 // 2 : (col0 + ncols_span) // 2]
                w = 32
                for lvl in range(1, 7):
                    r = cur.rearrange("p (blk two h) -> p blk two h", two=2, h=w)
                    if lvl < 6:
                        base = lvl_tiles[lvl - 1]
                        nxt = base[
                            :, (col0 // 128) * w : ((col0 + ncols_span) // 128) * w
                        ]
                        outap = nxt.rearrange("p (blk h) -> p blk h", h=w)
                    else:
                        nxt = None
                        outap = ost[
                            :, col0 // 128 : (col0 + ncols_span) // 128
                        ].rearrange("p (blk h) -> p blk h", h=1)
                    nc.vector.tensor_tensor(
                        out=outap, in0=r[:, :, 0, :], in1=r[:, :, 1, :],
                        op=_TREE_OPS[lvl],
                    )
                    cur = nxt
                    w //= 2

            for c, (col0, ln) in enumerate(chunks):
                ps = pss[c]
                for bank in range(ln // 512):
                    cb = col0 + bank * 512
                    nc.tensor.matmul(
                        ps[:, bass.ts(bank, 512)], w0t,
                        xslice(0, cb, 512), start=True, stop=False,
                    )
                for bank in range(ln // 512):
                    cb = col0 + bank * 512
                    nc.tensor.matmul(
                        ps[:, bass.ts(bank, 512)], w1t,
                        xslice(1, cb, 512), start=False, stop=True,
                    )
                evict(c)
                if c == 1:
                    tr_l1(nc.sync, 0, 2048)
                elif c == 3:
                    tr_l1(nc.sync, 2048, 2048)
                    tree(0, 4096)
                elif c == 5:
                    tr_l1(nc.sync, 4096, 2048)
                elif c == 7:
                    tr_l1(nc.sync, 6144, 1024)
                    tree(4096, 2048)
                elif c == 9:
                    tr_l1(nc.sync, 7168, 1024)
                    tree(6144, 2048)

            nc.sync.dma_start(out=out[:], in_=ost[:])

    nc.compile()
    return nc


_NC_CACHE: dict = {}


def _get_nc(bs=BS):
    if bs not in _NC_CACHE:
        _NC_CACHE[bs] = build_nc(bs)
    return _NC_CACHE[bs]


def prep_inputs(x: np.ndarray, W: np.ndarray, b: np.ndarray) -> list[dict]:
    perm = _bitrev7_perm()
    Wp = np.ascontiguousarray(W[:, perm]).astype(np.float16)
    bh = b[perm].astype(np.float16)
    x = np.asarray(x, dtype=np.float32)
    in_maps = []
    for i in range(NCORES):
        xi = x[i * BS : (i + 1) * BS, :].astype(np.float16)  # [8192, 256]
        xT = xi.T  # [256, 8192]
        xw0 = np.zeros((128, 4352), dtype=np.float16)
        xw1 = np.zeros((128, 4352), dtype=np.float16)
        xw0[:, 0:4096] = xT[0:128, 0:4096]
        xw0[:, 4096:4224] = Wp[0:128, :]
        xw0[:, 4224] = bh
        xw1[:, 0:4096] = xT[128:256, 0:4096]
        xw1[:, 4096:4224] = Wp[128:256, :]
        xr = np.ascontiguousarray(xT[:, 4096:8192].reshape(256, 4096))
        in_maps.append({"xw0": xw0, "xw1": xw1, "xr": xr})
    return in_maps


def gather_outputs(results: list[dict]) -> np.ndarray:
    shards = []
    for i in range(NCORES):
        o = np.asarray(results[i]["out"])  # [128, BS//128]; o[p, c] = row 128c+p
        shards.append(o.T.reshape(BS))
    return np.concatenate(shards).reshape(B, 1).astype(np.float32)


def _setup_tracing():
    """Install the antenv.axon_hooks NTFF-profile shim (missing from this
    image) and neuter the artifact upload so traced runs stay local."""
    import sys as _sys
    import types

    import concourse.bass_utils as bu

    bu.upload_artifacts = lambda tmpdir: tmpdir
    try:
        from antenv.axon_hooks import get_axon_ntff_profile_hook  # noqa: F401

        return
    except ImportError:
        pass
    import antenv

    m = types.ModuleType("antenv.axon_hooks")
    _state = {"hook": None}
    m.set_axon_ntff_profile_hook = lambda h: _state.__setitem__("hook", h)
    m.get_axon_ntff_profile_hook = lambda: _state["hook"]
    _sys.modules["antenv.axon_hooks"] = m
    antenv.axon_hooks = m
    try:
        from trn_agent_boot.trn_boot import _ntff_profile_via_ctypes

        hook = _ntff_profile_via_ctypes("/opt/axon/libaxon_pjrt.so")
        if hook is not None:
            m.set_axon_ntff_profile_hook(hook)
    except Exception as e:  # pragma: no cover
        print("ntff hook install failed:", e)


def run_on_hw(x, W, b, trace: bool = False, **kwargs):
    if trace:
        _setup_tracing()
    nc = _get_nc()
    in_maps = prep_inputs(np.asarray(x), np.asarray(W), np.asarray(b))
    return run_bass_kernel_spmd(
        nc, in_maps, core_ids=list(range(NCORES)), trace=trace, **kwargs
    )


def kernel(x: np.ndarray, W: np.ndarray, b: np.ndarray) -> np.ndarray:
    res = run_on_hw(x, W, b, trace=False)
    return gather_outputs(res.results)

